# revision 34
# baseline (speedup 1.0000x reference)
"""Trainium2 Bass kernel for nn_AAttnFusion (dual-stream area-attention fusion).

Sharding: 8 NeuronCores = (batch b = core//2) x (n-half j = core%2); each core
computes output rows {16t+8j+i : t in 0..3, i in 0..7} of its batch for all
256 channels -- fully local, zero collectives.

The reference's "area split" reshape [B,C,N]->[B*4,C,N/4] is row-major: it
splits the CHANNEL dim into 4 units (r) and remaps new-channel c2 to
(old channel 64r+c2//4, pixel quarter c2%4).  The permuted q/k layouts are
produced with DRAM-bounce gather DMAs; vT comes straight from x via
stationary-x matmuls; attention output is scattered back through DRAM.

The graph is j-agnostic (SPMD shares one NEFF): the host passes, per stream,
the full image (k-conv, vT), a row-gathered copy (q-conv) and a band-gathered
copy with halo rows + zero mask (dwconv), so no AP depends on the core id.

Matmuls run in float32r (tf32, 1 cyc/row at N>=256; host pre-rounds weights
and inputs) or bf16 (attention AV/sums, dwconv taps).  Softmax skips the
max-subtract (|logits| < ~4 for this problem); denominators come from a
ones-lhsT matmul packed into spare PE col-strips; 1/s = exp(-ln(s)) keeps the
whole attention phase in one ACT table set (natural_log_exp_and_others).
"""
import sys

for _p in ("/opt/trn_rl_repo",):
    if _p not in sys.path:
        sys.path.insert(0, _p)

import numpy as np
import ml_dtypes

B, CIN, H, W, DIM = 4, 256, 64, 64, 128
N = H * W
SCALE = float(32.0 ** -0.5)
ND = 4 * 14 * 64  # dwconv-copy pixel count (4 bands x 14 rows x 64)

_CACHE = {}


def _tf32(a):
    a = np.ascontiguousarray(a, dtype=np.float32)
    return (a.view(np.uint32) & np.uint32(0xFFFFE000)).view(np.float32)


def _bf16(a):
    return np.ascontiguousarray(a, dtype=np.float32).astype(ml_dtypes.bfloat16)


def _build():
    import concourse.bacc as bacc
    import concourse.mybir as mybir
    from concourse.tile import TileContext

    f32 = mybir.dt.float32
    f32r = mybir.dt.float32r
    bf16 = mybir.dt.bfloat16
    AF = mybir.ActivationFunctionType
    ALU = mybir.AluOpType

    nc = bacc.Bacc("TRN2", target_bir_lowering=False, debug=False, num_devices=8)

    P = {}

    def par(name, shape, dt):
        P[name] = nc.declare_dram_parameter(name, shape, dt, isOutput=False)

    for s in ("r", "c"):
        par(s + "_in", [2, 128, N], bf16)        # full image, channel k-tiles
        par(s + "_inq", [2, 128, 2048], bf16)    # rows {16t+8j+i}, (t,i,x) order
        par(s + "_ind", [2, 128, ND], bf16)      # dwconv bands (4 x 14 rows)
        par(s + "_w1Tb", [2, 128, 128], bf16)    # bf16 conv1 weights (dwconv path)
        par(s + "_wvTb", [128, 128], bf16)
        par(s + "_wqTb", [128, 128], bf16)
        par(s + "_wkTb", [128, 128], bf16)
        par(s + "_mask", [128, 6 * 64], bf16)    # 1/0 for the 6 edge rows only
        par(s + "_w1T", [2, 128, 128], f32r)
        par(s + "_b1", [128, 1], f32)
        for w in ("wqT", "wkT", "wvT", "wpT"):
            par(s + "_" + w, [128, 128], f32r)
        for bias in ("bq", "bk", "bv", "bp"):
            par(s + "_" + bias, [128, 1], f32)
        par(s + "_dw", [128, 49, 128], bf16)     # diag 7x7 taps as [K, tap, M]
        par(s + "_dwv", [128, 49], f32)          # tap weights per-partition (DVE)
    par("f_wT", [128, 4, 128], f32r)             # fuse lhsT blocks [ki, mo*2+ki]
    par("f_wTb", [128, 4, 128], bf16)
    par("f_b", [128, 2], f32)
    par("ones_bf", [128, 32], bf16)
    par("ones_fr", [128, 32], f32r)
    out_p = nc.declare_dram_parameter("out", [2, 128, 2048], f32, isOutput=True)

    with TileContext(nc) as tc:
        with tc.tile_pool(name="wpool", bufs=1) as wp, \
             tc.tile_pool(name="dram", bufs=1, space="DRAM") as dp, \
             tc.tile_pool(name="big", bufs=1) as bp, \
             tc.tile_pool(name="ld", bufs=2) as ldp, \
             tc.tile_pool(name="cyc", bufs=2) as cp:

            # ---- load weights/biases (phase-A-critical ones only; rest deferred) ----
            Wt = {}

            def loadw(s, wname, shape, dt):
                t = wp.tile(shape, dt, name=f"{s}_{wname}_sb")
                src = P[f"{s}_{wname}"][:]
                if wname in ("w1T", "w1Tb"):
                    src = src.rearrange("k p m -> p k m")
                nc.sync.dma_start(out=t[:], in_=src)
                Wt[f"{s}_{wname}"] = t

            for s in ("r", "c"):
                for wname, shape, dt in (
                    ("w1Tb", [128, 2, 128], bf16), ("wqTb", [128, 128], bf16),
                    ("wkTb", [128, 128], bf16), ("wvTb", [128, 128], bf16),
                    ("wvT", [128, 128], f32r),
                    ("b1", [128, 1], f32), ("bq", [128, 1], f32),
                    ("bk", [128, 1], f32), ("bv", [128, 1], f32),
                ):
                    loadw(s, wname, shape, dt)
            for wname, shape, dt in (("ones_bf", [128, 32], bf16), ("ones_fr", [128, 32], f32r)):
                t = wp.tile(shape, dt, name=wname + "_sb")
                nc.sync.dma_start(out=t[:], in_=P[wname][:])
                Wt[wname] = t

            def load_late_weights():
                for s in ("r", "c"):
                    for wname, shape, dt in (
                        ("wpT", [128, 128], f32r), ("dw", [128, 49, 128], bf16),
                        ("dwv", [128, 49], f32),
                        ("bp", [128, 1], f32), ("mask", [128, 6 * 64], bf16),
                    ):
                        loadw(s, wname, shape, dt)
                for wname, shape, dt in (("f_wTb", [128, 4, 128], bf16), ("f_b", [128, 2], f32)):
                    t = wp.tile(shape, dt, name=wname + "_sb")
                    nc.sync.dma_start(out=t[:], in_=P[wname][:])
                    Wt[wname] = t

            def cmm(ps, lhsT, rhs, start=True, stop=True):
                # 4-way col-tiled conv matmul: concurrent strips, 4x less
                # serialized LDWEIGHTS exposure
                for c4 in range(4):
                    nc.tensor.matmul(ps[32 * c4:32 * c4 + 32, :],
                                     lhsT[:, 32 * c4:32 * c4 + 32], rhs,
                                     start=start, stop=stop, tile_position=(0, 32 * c4))

            # DRAM scratch for gathers/scatters
            qo_d = {s: dp.tile([128, 2048], bf16, name=f"{s}_qo_d") for s in ("r", "c")}
            ko_d = {s: dp.tile([128, 4096], bf16, name=f"{s}_ko_d") for s in ("r", "c")}
            attU_d = {s: dp.tile([128, 2048], f32, name=f"{s}_attU_d") for s in ("r", "c")}
            attS_d = {s: dp.tile([128, 2048], f32, name=f"{s}_attS_d") for s in ("r", "c")}

            v4p = {}
            vt = {}
            g = {}

            # =============== Phase A: convs + vT (both streams) ===============
            # A_early: x, k-conv, q-conv, vT (attention prerequisites)
            # A_late:  xd, v-conv, v4p (dwconv inputs; only gate phase C)
            with tc.tile_pool(name="psA", bufs=4, space="PSUM") as psA:
                for s in ("r", "c"):
                    w1T, wqT, wkT, wvT = (Wt[f"{s}_{k}"] for k in ("w1Tb", "wqTb", "wkTb", "wvTb"))
                    b1, bq, bk, bv = (Wt[f"{s}_{k}"] for k in ("b1", "bq", "bk", "bv"))

                    # x on full image
                    x = bp.tile([128, 4096], bf16, name="x", tag="x")
                    for ch in range(8):
                        ld = ldp.tile([128, 2, 512], bf16, name="ld_in", tag="ld", bufs=8)
                        nc.sync.dma_start(out=ld[:], in_=P[s + "_in"][:, :, ch * 512:(ch + 1) * 512]
                                          .rearrange("k p n -> p k n"))
                        ps = psA.tile([128, 512], f32, name="psc", tag="psc")
                        nc.tensor.matmul(ps[:], w1T[:, 0, :], ld[:, 0, :], start=True, stop=False)
                        nc.tensor.matmul(ps[:], w1T[:, 1, :], ld[:, 1, :], start=False, stop=True)
                        nc.scalar.activation(x[:, ch * 512:(ch + 1) * 512], ps[:], AF.Silu, bias=b1[:])
                    # k conv (full image) -> DRAM ko_d
                    for ch in range(8):
                        ps = psA.tile([128, 512], f32, name="psc", tag="psc")
                        nc.tensor.matmul(ps[:], wkT[:], x[:, ch * 512:(ch + 1) * 512], start=True, stop=True)
                        kc = cp.tile([128, 512], bf16, name="kc", tag="kc", bufs=4)
                        nc.scalar.activation(kc[:], ps[:], AF.Identity, bias=bk[:])
                        nc.sync.dma_start(out=ko_d[s][:, ch * 512:(ch + 1) * 512], in_=kc[:])
                    # x_q (host-gathered rows) + q conv -> DRAM qo_d
                    xq = bp.tile([128, 2048], bf16, name="xq", tag="xq")
                    for ch in range(4):
                        ld = ldp.tile([128, 2, 512], bf16, name="ld_inq", tag="ld", bufs=8)
                        nc.sync.dma_start(out=ld[:], in_=P[s + "_inq"][:, :, ch * 512:(ch + 1) * 512]
                                          .rearrange("k p n -> p k n"))
                        ps = psA.tile([128, 512], f32, name="psc", tag="psc")
                        nc.tensor.matmul(ps[:], w1T[:, 0, :], ld[:, 0, :], start=True, stop=False)
                        nc.tensor.matmul(ps[:], w1T[:, 1, :], ld[:, 1, :], start=False, stop=True)
                        nc.scalar.activation(xq[:, ch * 512:(ch + 1) * 512], ps[:], AF.Silu, bias=b1[:])
                    for ch in range(4):
                        ps = psA.tile([128, 512], f32, name="psc", tag="psc")
                        nc.tensor.matmul(ps[:], wqT[:], xq[:, ch * 512:(ch + 1) * 512], start=True, stop=True)
                        qc = cp.tile([128, 512], bf16, name="qc", tag="kc", bufs=4)
                        nc.scalar.activation(qc[:], ps[:], AF.Identity, bias=bq[:])
                        nc.sync.dma_start(out=qo_d[s][:, ch * 512:(ch + 1) * 512], in_=qc[:])
                    # vT via stationary-x matmuls; one matmul covers ALL 4 r-units:
                    # out[m_px, 32r+u] = sum_cin x[cin, t*1024+mt*128+m] Wv[32r+u, cin]
                    # vt_s [128 m, 8 mt, 512 (r*128 + c2v)] bf16; c2v = 4u+t
                    v = bp.tile([128, 8, 1024], bf16, name=f"vt_{s}")
                    vt[s] = v
                    # ones columns (denominator rides in the AV matmul)
                    nc.vector.memset(
                        v[:].rearrange("p m (blk c) -> p (m blk) c", c=64)[:, :, 32:64], 1.0)
                    for t in range(4):
                        for mt in range(8):
                            psv = psA.tile([128, 128], f32, name="psv", tag="psv", bufs=2)
                            nc.tensor.matmul(
                                psv[:],
                                x[:, t * 1024 + mt * 128: t * 1024 + (mt + 1) * 128],
                                wvT[:], start=True, stop=True)
                            # reorder cols (r, u=8h+u2) -> r*256 + h*64 + 4*u2 + t
                            nc.vector.tensor_copy(
                                out=v[:, mt, :].rearrange("p (r h q u2 t4) -> p r h q u2 t4",
                                                          r=4, h=4, q=2, u2=8)[:, :, :, 0, :, t],
                                in_=psv[:].rearrange("p (r h u2) -> p r h u2", r=4, h=4))

                load_late_weights()

                # A_late: dwconv inputs
                for s in ("r", "c"):
                    w1T, wvT = Wt[s + "_w1Tb"], Wt[s + "_wvTb"]
                    b1, bv = Wt[s + "_b1"], Wt[s + "_bv"]
                    xd = bp.tile([128, ND], bf16, name="xd", tag="xd")
                    for ch in range(8):
                        ld = ldp.tile([128, 2, 448], bf16, name="ld_ind", tag="ldb", bufs=4)
                        nc.sync.dma_start(out=ld[:], in_=P[s + "_ind"][:, :, ch * 448:(ch + 1) * 448]
                                          .rearrange("k p n -> p k n"))
                        ps = psA.tile([128, 448], f32, name="psd", tag="psc")
                        cmm(ps, w1T[:, 0, :], ld[:, 0, :], start=True, stop=False)
                        cmm(ps, w1T[:, 1, :], ld[:, 1, :], start=False, stop=True)
                        nc.scalar.activation(xd[:, ch * 448:(ch + 1) * 448], ps[:], AF.Silu, bias=b1[:])
                    vp = bp.tile([128, 56, 70], bf16, name=f"v4p_{s}")
                    v4p[s] = vp
                    nc.vector.memset(vp[:], 0.0)
                    for ch in range(8):
                        ps = psA.tile([128, 448], f32, name="psd", tag="psc")
                        cmm(ps, wvT[:], xd[:, ch * 448:(ch + 1) * 448])
                        nc.scalar.activation(
                            vp[:, ch * 7:(ch + 1) * 7, 3:67],
                            ps[:].rearrange("p (r w) -> p r w", w=64), AF.Identity, bias=bv[:])
                    # zero out-of-image edge rows: band0 rows 0..2, band3 rows 11..13
                    msk = Wt[s + "_mask"][:].rearrange("p (r w) -> p r w", w=64)
                    nc.vector.tensor_tensor(
                        out=vp[:, 0:3, 3:67], in0=vp[:, 0:3, 3:67],
                        in1=msk[:, 0:3, :], op=ALU.mult)
                    nc.vector.tensor_tensor(
                        out=vp[:, 53:56, 3:67], in0=vp[:, 53:56, 3:67],
                        in1=msk[:, 3:6, :], op=ALU.mult)

            # =============== Phase B+C: attention, dwconv, proj ===============
            with tc.tile_pool(name="psB", bufs=1, space="PSUM") as psB:
                def attention(s):
                    for r in range(4):
                        qr = cp.tile([128, 512], bf16, name="qr", tag="qr", bufs=2)
                        nc.sync.dma_start(out=qr[:], in_=qo_d[s][32 * r:32 * r + 32, :]
                                          .rearrange("p (t n) -> (p t) n", t=4))
                        kr = cp.tile([128, 1024], bf16, name="kr", tag="kr", bufs=2)
                        nc.sync.dma_start(out=kr[:], in_=ko_d[s][32 * r:32 * r + 32, :]
                                          .rearrange("p (t n) -> (p t) n", t=4))
                        av_ps = psB.tile([128, 1024], f32, name="av", tag="av")

                        def av_sums(mt, pth):
                            # merged AV+denominator: lhsT = [vT_hh | ones], M=64
                            for hh in range(4):
                                nc.tensor.matmul(
                                    av_ps[64 * (hh % 2):64 * (hh % 2) + 64,
                                          (hh // 2) * 512:(hh // 2) * 512 + 512],
                                    vt[s][:, mt, r * 256 + 64 * hh: r * 256 + 64 * hh + 64],
                                    pth[hh // 2][:, (hh % 2) * 512:(hh % 2) * 512 + 512],
                                    start=(mt == 0), stop=(mt == 7), tile_position=(0, 64 * (hh % 2)))

                        # software pipeline: 2-bank qk halves so exp_A overlaps QK_B;
                        # AV/sums of mt-1 run on PE while ACT exps mt
                        prev = None
                        for mt in range(8):
                            pth = []
                            for half, tag in ((0, "qka"), (1, "qkb")):
                                qk = psB.tile([128, 1024], f32, name=tag, tag=tag)
                                for hi in range(2):
                                    hh = 2 * half + hi
                                    nc.tensor.matmul(
                                        qk[:, hi * 512:(hi + 1) * 512],
                                        kr[32 * hh:32 * hh + 32, mt * 128:(mt + 1) * 128],
                                        qr[32 * hh:32 * hh + 32, :],
                                        start=True, stop=True, tile_position=(32 * hh, 0))
                                pt = cp.tile([128, 1024], bf16, name="pt" + tag, tag="pt" + tag, bufs=3)
                                nc.scalar.activation(pt[:], qk[:], AF.Exp, scale=SCALE)
                                pth.append(pt)
                            if prev is not None:
                                av_sums(*prev)
                            prev = (mt, pth)
                        av_sums(*prev)
                        # scatter unnormalized AV and denominator fields to DRAM
                        avsb = cp.tile([128, 1024], f32, name="avsb", tag="avsb", bufs=2)
                        nc.vector.tensor_copy(out=avsb[:], in_=av_ps[:])
                        for hh in range(4):
                            pb, cb = 64 * (hh % 2), (hh // 2) * 512
                            nc.sync.dma_start(
                                out=attU_d[s][32 * r + 8 * hh: 32 * r + 8 * hh + 8, :]
                                .rearrange("p (t n) -> (p t) n", t=4),
                                in_=avsb[pb:pb + 32, cb:cb + 512])
                            nc.sync.dma_start(
                                out=attS_d[s][32 * r + 8 * hh: 32 * r + 8 * hh + 8, :]
                                .rearrange("p (t n) -> (p t) n", t=4),
                                in_=avsb[pb + 32:pb + 64, cb:cb + 512])

                # ---- dwconv taps + proj + gelu (per stream) ----
                def phase_c(s):
                    attU = bp.tile([128, 2048], f32, name="attU", tag="attU")
                    nc.sync.dma_start(out=attU[:], in_=attU_d[s][:])
                    attS = bp.tile([128, 2048], f32, name="attS", tag="attS")
                    nc.sync.dma_start(out=attS[:], in_=attS_d[s][:])
                    rin = bp.tile([128, 2048], f32, name="rin", tag="rin")
                    nc.vector.reciprocal_approx_fast(out=rin[:], in_=attS[:])
                    attin = bp.tile([128, 2048], f32r, name="attin", tag="attin")
                    nc.vector.tensor_tensor(out=attin[:], in0=attU[:], in1=rin[:], op=ALU.mult)
                    gs = bp.tile([128, 2048], bf16, name=f"g_{s}")
                    g[s] = gs
                    for t in range(4):
                        # odd-dx taps on PE (diag matmuls)
                        pe_taps = [kk for kk in range(49) if (kk % 7) not in (0, 2, 4)]
                        dve_taps = [kk for kk in range(49) if (kk % 7) in (0, 2, 4)]
                        pp = psB.tile([128, 512], f32, name="pp", tag="pp")
                        for i, kk in enumerate(pe_taps):
                            dy, dx = kk // 7, kk % 7
                            nc.tensor.matmul(
                                pp[:],
                                Wt[s + "_dw"][:, kk, :],
                                v4p[s][:, t * 14 + dy: t * 14 + dy + 8, dx:dx + 64],
                                start=(i == 0), stop=(i == len(pe_taps) - 1))
                        # even-dx taps on DVE (4B-aligned bf16 windows -> 2x mode)
                        ppd = cp.tile([128, 512], bf16, name="ppd", tag="ppd", bufs=2)
                        for i, kk in enumerate(dve_taps):
                            dy, dx = kk // 7, kk % 7
                            win = v4p[s][:, t * 14 + dy: t * 14 + dy + 8, dx:dx + 64]
                            sc = Wt[s + "_dwv"][:, kk:kk + 1]
                            if i == 0:
                                nc.vector.tensor_scalar_mul(ppd[:].rearrange("p (r w) -> p r w", w=64),
                                                            win, sc)
                            else:
                                nc.vector.scalar_tensor_tensor(
                                    out=ppd[:].rearrange("p (r w) -> p r w", w=64),
                                    in0=win, scalar=sc,
                                    in1=ppd[:].rearrange("p (r w) -> p r w", w=64),
                                    op0=ALU.mult, op1=ALU.add)
                        projin = cp.tile([128, 512], f32r, name="projin", tag="projin", bufs=2)
                        nc.vector.tensor_tensor(out=projin[:], in0=pp[:],
                                                in1=attin[:, t * 512:(t + 1) * 512], op=ALU.add)
                        nc.vector.tensor_tensor(out=projin[:], in0=projin[:],
                                                in1=ppd[:], op=ALU.add)
                        ps = psB.tile([128, 512], f32, name="prj", tag="prj")
                        nc.tensor.matmul(ps[:], Wt[s + "_wpT"][:], projin[:], start=True, stop=True)
                        nc.scalar.activation(gs[:, t * 512:(t + 1) * 512], ps[:], AF.Gelu,
                                             bias=Wt[s + "_bp"][:])
                    if s == "c":
                        for t in range(4):
                            for mo in range(2):
                                fp = psB.tile([128, 512], f32, name="fps", tag="prj")
                                nc.tensor.matmul(fp[:], Wt["f_wTb"][:, 2 * mo + 0, :],
                                                 g["r"][:, t * 512:(t + 1) * 512], start=True, stop=False)
                                nc.tensor.matmul(fp[:], Wt["f_wTb"][:, 2 * mo + 1, :],
                                                 gs[:, t * 512:(t + 1) * 512], start=False, stop=True)
                                ob = cp.tile([128, 512], f32, name="ob", tag="ob")
                                nc.scalar.activation(ob[:], fp[:], AF.Silu, bias=Wt["f_b"][:, mo:mo + 1])
                                nc.sync.dma_start(out=out_p[mo, :, t * 512:(t + 1) * 512], in_=ob[:])

                attention("r")
                phase_c("r")
                attention("c")
                phase_c("c")


    nc.compile()
    return nc


def _rows_for(j):
    return np.concatenate([np.arange(16 * t + 8 * j, 16 * t + 8 * j + 8) for t in range(4)])


def _host_inputs(inputs):
    """Build the 8 per-core input maps."""
    per_stream = {}
    for s, img_key, pre in (("r", "rgb", "rgb1"), ("c", "chm", "chm1")):
        qkv_w = inputs[f"{s}_qkv_w"] if f"{s}_qkv_w" in inputs else None
        per_stream[s] = dict(
            img=np.ascontiguousarray(inputs[img_key], np.float32),
            w1=inputs[f"{pre}_w"], b1=inputs[f"{pre}_b"],
            qkv_w=inputs[f"{s}_qkv_w"], qkv_b=inputs[f"{s}_qkv_b"],
            pe_w=inputs[f"{s}_pe_w"], pe_b=inputs[f"{s}_pe_b"],
            proj_w=inputs[f"{s}_proj_w"], proj_b=inputs[f"{s}_proj_b"],
        )

    const = {}
    q_rows_sel = np.concatenate([np.arange(64 * r, 64 * r + 32) for r in range(4)])
    k_rows_sel = np.concatenate([np.arange(64 * r + 32, 64 * r + 64) for r in range(4)])
    for s in ("r", "c"):
        d = per_stream[s]
        w1 = np.asarray(d["w1"], np.float32)          # [128, 256]
        qkv_w = np.asarray(d["qkv_w"], np.float32)    # [384, 128]
        qkv_b = np.asarray(d["qkv_b"], np.float32)
        pe_w = np.asarray(d["pe_w"], np.float32)      # [128, 1, 7, 7]
        pe_b = np.asarray(d["pe_b"], np.float32)
        proj_w = np.asarray(d["proj_w"], np.float32)
        proj_b = np.asarray(d["proj_b"], np.float32)
        Wv = qkv_w[256:384]
        bv = qkv_b[256:384]
        const[s + "_w1T"] = _tf32(w1.T.reshape(256, 128).reshape(2, 128, 128))
        const[s + "_b1"] = np.asarray(d["b1"], np.float32).reshape(128, 1)
        const[s + "_wqT"] = _tf32(qkv_w[q_rows_sel].T)
        const[s + "_bq"] = qkv_b[q_rows_sel].reshape(128, 1)
        const[s + "_wkT"] = _tf32(qkv_w[k_rows_sel].T)
        const[s + "_bk"] = qkv_b[k_rows_sel].reshape(128, 1)
        const[s + "_wvT"] = _tf32(Wv.T)
        const[s + "_w1Tb"] = _bf16(w1.T.reshape(256, 128).reshape(2, 128, 128))
        const[s + "_wvTb"] = _bf16(Wv.T)
        const[s + "_wqTb"] = _bf16(qkv_w[q_rows_sel].T)
        const[s + "_wkTb"] = _bf16(qkv_w[k_rows_sel].T)
        const[s + "_bv"] = bv.reshape(128, 1)
        const[s + "_wpT"] = _tf32(proj_w.T)
        const[s + "_bp"] = (proj_b + proj_w @ bv + proj_w @ pe_b).reshape(128, 1)
        dw = np.zeros((128, 49, 128), np.float32)
        kidx = np.arange(128)
        for kk in range(49):
            dw[kidx, kk, kidx] = pe_w[:, 0, kk // 7, kk % 7]
        const[s + "_dw"] = _bf16(dw)
        const[s + "_dwv"] = np.ascontiguousarray(
            pe_w[:, 0].reshape(128, 49), np.float32)
    fuse_w = np.asarray(inputs["fuse_w"], np.float32)
    fwT = np.zeros((128, 4, 128), np.float32)
    FT = fuse_w.T  # [256 cin, 256 cout]
    for mo in range(2):
        for ki in range(2):
            fwT[:, 2 * mo + ki, :] = FT[ki * 128:(ki + 1) * 128, mo * 128:(mo + 1) * 128]
    const["f_wT"] = _tf32(fwT)
    const["f_wTb"] = _bf16(fwT)
    const["f_b"] = np.asarray(inputs["fuse_b"], np.float32).reshape(2, 128).T.copy()
    const["ones_bf"] = _bf16(np.ones((128, 32), np.float32))
    const["ones_fr"] = _tf32(np.ones((128, 32), np.float32))

    in_maps = []
    for core in range(8):
        b, j = core // 2, core % 2
        m = dict(const)
        rows = _rows_for(j)
        for s in ("r", "c"):
            img = per_stream[s]["img"][b].reshape(256, H, W)  # [256, 64, 64]
            m[s + "_in"] = _bf16(img.reshape(2, 128, N))
            m[s + "_inq"] = _bf16(img[:, rows, :].reshape(2, 128, 2048))
            ind = np.zeros((256, 4, 14, 64), np.float32)
            msk = np.zeros((4, 14, 64), np.float32)
            for t in range(4):
                lo = 16 * t + 8 * j - 3
                for w in range(14):
                    gr = lo + w
                    if 0 <= gr < H:
                        ind[:, t, w, :] = img[:, gr, :]
                        msk[t, w, :] = 1.0
            m[s + "_ind"] = _bf16(ind.reshape(2, 128, ND))
            edge = np.stack([msk[0, 0], msk[0, 1], msk[0, 2],
                             msk[3, 11], msk[3, 12], msk[3, 13]])  # [6, 64]
            m[s + "_mask"] = _bf16(np.broadcast_to(edge.reshape(1, 6 * 64), (128, 6 * 64)))
        in_maps.append(m)
    return in_maps


def kernel(**inputs):
    from concourse.bass_utils import run_bass_kernel_spmd

    if "nc" not in _CACHE:
        _CACHE["nc"] = _build()
    nc = _CACHE["nc"]
    in_maps = _host_inputs(inputs)
    res = run_bass_kernel_spmd(nc, in_maps, core_ids=list(range(8)))
    full = np.zeros((B, CIN, H, W), np.float32)
    for core in range(8):
        b, j = core // 2, core % 2
        rows = _rows_for(j)
        o = res.results[core]["out"].reshape(256, 4, 8, 64)  # [ch, t, i, x]
        for t in range(4):
            full[b, :, 16 * t + 8 * j:16 * t + 8 * j + 8, :] = o[:, t, :, :]
    return full
